# revision 15
# baseline (speedup 1.0000x reference)
"""KNN top-32 kernel for Trainium2 (Bass/Tile), 8 NeuronCores.

Strategy:
  - Data-parallel over batch: core b handles batch element b (M=4096 queries,
    N=16384 database points, C=3).
  - Host ships raw coords only: qa3 = [2*q | x] as [3, M+N] f32 per core
    (1.97 MB total H2D). No augmented row is materialized: per query tile the
    PE computes s = 2*q.x - |x|^2 (a monotone-decreasing transform of the
    squared distance, per query row) as TWO accumulating K=3 fp32 matmuls
    into one PSUM tile: lhsT=(2q) (3 x 128) @ x (3 x 512), then
    lhsT=(-1s) (3 x 128) @ (x*x) (3 x 512), which adds -|x_n|^2 to every
    query row.
  - DVE reduces each 512-chunk with max8 (top-8 values) + max_index (their
    in-chunk indices) straight out of PSUM into a 256-wide table per 128-query
    tile. The true top-32 of a row is contained in the per-segment top-8 table
    (any segment holds at most 8 of a row's top-32 with overwhelming
    probability for randn inputs).
  - NROUNDS rounds of max8+match_replace(-BIG) on the table mark the top
    NCAND table slots; a compare+multiply turns the paired index table into a
    sparse key array (global_idx+1 at winners, 0 elsewhere), and NROUNDS more
    max8+match_replace rounds compact the candidate indices out, order-free.
    Output as uint16 (global_idx+1 fits 16 bits) to halve the D2H fetch.
  - Host re-ranks the candidates per query with bit-exact f32 reference
    arithmetic (single uint64 sort key: monotone f32 bits << 14 | idx) and
    emits the top-32 indices (int32).

Perf notes (axon-tunneled cores; wall-clock is network-dominated):
  - The jitted shard_map executable is built ONCE and cached; per-call cost is
    H2D of qa3, the NEFF execute (~ms), D2H of the u16 candidates.
  - Per-shard D2H fetches run in worker threads concurrently, and each batch
    element's rerank runs as its shard lands, hiding rerank behind transfers.
  - The per-segment offset table is generated on device with iota (no H2D).
  - The donated output buffers are created on device (no zeros upload).
"""

import numpy as np
from concurrent.futures import ThreadPoolExecutor

import jax
import jax.numpy as jnp
from jax.sharding import Mesh, NamedSharding, PartitionSpec
from jax.experimental.shard_map import shard_map

import concourse.bass as bass
from concourse import bacc, bass2jax
import concourse.mybir as mybir
from concourse.tile import TileContext

B = 8
M = 4096          # queries per core
N = 16384         # database points per core
K = 32            # neighbors wanted
NROUNDS = 4
NCAND = 8 * NROUNDS  # candidates extracted per query
SEG = 512
G = N // SEG      # 32 segments -> table width 256
TW = G * 8        # table width
MT = 128          # query rows per tile
NT = M // MT      # 32 row tiles
NEG = -1.0e30

F32 = mybir.dt.float32
I32 = mybir.dt.int32
U16 = mybir.dt.uint16


def build_bass():
    nc = bacc.Bacc()
    qa3 = nc.declare_dram_parameter("qa3", [3, M + N], F32, isOutput=False)
    out = nc.declare_dram_parameter("out", [M, NCAND], U16, isOutput=True)

    with TileContext(nc) as tc, \
         tc.tile_pool(name="const", bufs=1) as cpool, \
         tc.tile_pool(name="work", bufs=2) as wpool, \
         tc.tile_pool(name="outp", bufs=3) as opool, \
         tc.tile_pool(name="psum", bufs=8, space="PSUM") as ppool:
        qx = cpool.tile([3, M + N], F32)
        nc.sync.dma_start(out=qx[:, :], in_=qa3[:, :])
        qs = qx[:, :M]
        asb = qx[:, M:]
        # database squares (for the -|x|^2 accumulation matmul)
        sq = cpool.tile([3, N], F32)
        nc.vector.tensor_mul(sq[:, :], asb[:, :], asb[:, :])
        # [3, MT] of -1: second matmul adds Sum_c -sq[c,n] = -|x_n|^2 to every
        # query row of the PSUM tile
        neg1 = cpool.tile([3, MT], F32)
        nc.vector.memset(neg1[:, :], -1.0)

        # per-slot global offset table: slot j -> (j//8)*SEG + 1 (same for all
        # partitions), generated on device instead of shipped over the tunnel
        ioff = cpool.tile([MT, TW], I32)
        nc.gpsimd.iota(ioff[:, :], [[SEG, G], [0, 8]], base=1,
                       channel_multiplier=0)
        offt = cpool.tile([MT, TW], F32)
        nc.vector.tensor_copy(offt[:, :], ioff[:, :])

        for t in range(NT):
            tbl = wpool.tile([MT, TW], F32, tag="tbl")
            idx16 = wpool.tile([MT, TW], U16, tag="idx16")
            for g in range(G):
                ps = ppool.tile([MT, SEG], F32, tag="ps")
                nc.tensor.matmul(
                    ps[:, :],
                    qs[:, t * MT:(t + 1) * MT],
                    asb[:, g * SEG:(g + 1) * SEG],
                    start=True,
                    stop=False,
                )
                nc.tensor.matmul(
                    ps[:, :],
                    neg1[:, :],
                    sq[:, g * SEG:(g + 1) * SEG],
                    start=False,
                    stop=True,
                )
                nc.vector.max(out=tbl[:, g * 8:(g + 1) * 8], in_=ps[:, :])
                nc.vector.max_index(
                    out=idx16[:, g * 8:(g + 1) * 8],
                    in_max=tbl[:, g * 8:(g + 1) * 8],
                    in_values=ps[:, :],
                )
            # paired global index table (value = global idx + 1) as f32
            idxf = wpool.tile([MT, TW], F32, tag="idxf")
            nc.vector.tensor_copy(idxf[:, :], idx16[:, :])
            nc.vector.tensor_add(idxf[:, :], idxf[:, :], offt[:, :])
            # pop top-NCAND values; winners' slots become NEG
            v8 = wpool.tile([MT, 8], F32, tag="v8")
            for r in range(NROUNDS):
                nc.vector.max(out=v8[:, :], in_=tbl[:, :])
                nc.vector.match_replace(
                    out=tbl[:, :], in_to_replace=v8[:, :], in_values=tbl[:, :],
                    imm_value=NEG,
                )
            # sparse key array: idx+1 where popped, 0 elsewhere
            wmask = wpool.tile([MT, TW], F32, tag="wmask")
            nc.vector.tensor_scalar(
                wmask[:, :], tbl[:, :], NEG, None, op0=mybir.AluOpType.is_equal
            )
            key = wpool.tile([MT, TW], F32, tag="key")
            nc.vector.tensor_mul(key[:, :], wmask[:, :], idxf[:, :])
            # compact the NCAND winning indices (order-free)
            outt = opool.tile([MT, NCAND], F32, tag="outt")
            for r in range(NROUNDS):
                nc.vector.max(out=outt[:, r * 8:(r + 1) * 8], in_=key[:, :])
                if r < NROUNDS - 1:
                    nc.vector.match_replace(
                        out=key[:, :], in_to_replace=outt[:, r * 8:(r + 1) * 8],
                        in_values=key[:, :], imm_value=0.0,
                    )
            out16 = opool.tile([MT, NCAND], U16, tag="out16")
            nc.vector.tensor_copy(out16[:, :], outt[:, :])
            nc.sync.dma_start(out=out[t * MT:(t + 1) * MT, :], in_=out16[:, :])
    nc.finalize()
    return nc


class _State:
    __slots__ = ("nc", "sharded", "zeros_jit", "pool", "next_zeros")


_STATE = None


def _get_state():
    global _STATE
    if _STATE is not None:
        return _STATE
    nc = build_bass()
    bass2jax.install_neuronx_cc_hook()

    partition_name = nc.partition_id_tensor.name if nc.partition_id_tensor else None
    in_names, out_names, out_avals = [], [], []
    for alloc in nc.m.functions[0].allocations:
        if not isinstance(alloc, mybir.MemoryLocationSet):
            continue
        name = alloc.memorylocations[0].name
        if alloc.kind == "ExternalInput":
            if name != partition_name:
                in_names.append(name)
        elif alloc.kind == "ExternalOutput":
            out_names.append(name)
            out_avals.append(jax.core.ShapedArray(
                tuple(alloc.tensor_shape), mybir.dt.np(alloc.dtype)))
    assert in_names == ["qa3"] and out_names == ["out"], (in_names, out_names)
    all_names = list(in_names + out_names)
    if partition_name is not None:
        all_names.append(partition_name)
    n_params = len(in_names)

    def _body(*args):
        operands = list(args)
        if partition_name is not None:
            operands.append(bass2jax.partition_id_tensor())
        outs = bass2jax._bass_exec_p.bind(
            *operands,
            out_avals=tuple(out_avals),
            in_names=tuple(all_names),
            out_names=tuple(out_names),
            lowering_input_output_aliases=(),
            sim_require_finite=True,
            sim_require_nnan=True,
            nc=nc,
        )
        return tuple(outs)

    devices = jax.devices()[:B]
    assert len(devices) == B, f"need {B} devices, got {len(jax.devices())}"
    mesh = Mesh(np.asarray(devices), ("core",))
    spec = PartitionSpec("core")
    sharded = jax.jit(
        shard_map(_body, mesh=mesh, in_specs=(spec,) * (n_params + 1),
                  out_specs=(spec,), check_rep=False),
        donate_argnums=(n_params,),
        keep_unused=True,
    )
    zeros_jit = jax.jit(
        lambda: jnp.zeros((B * M, NCAND), jnp.uint16),
        out_shardings=NamedSharding(mesh, spec),
    )
    st = _State()
    st.nc = nc
    st.sharded = sharded
    st.zeros_jit = zeros_jit
    st.pool = ThreadPoolExecutor(B)
    st.next_zeros = zeros_jit()
    _STATE = st
    return st


def _prep_qa3(xyz, new_xyz):
    """Concatenated per-core raw-coord matrices: [B*3, M+N] f32.

    Row block b: [2*qx, 2*qy, 2*qz | x, y, z] for batch element b. The query
    side carries the factor 2 (exact in fp32) so the device matmul computes
    s = 2*q.x - |x|^2 with the same rounding as scaling the database side.
    """
    qa3 = np.empty((B, 3, M + N), np.float32)
    np.multiply(new_xyz.transpose(0, 2, 1), np.float32(2.0), out=qa3[:, :, :M])
    qa3[:, :, M:] = xyz.transpose(0, 2, 1)
    return qa3.reshape(B * 3, M + N)


def _rerank_one(cand_u16, xyz_b, q, qn, q64):
    """cand_u16: [m, NCAND] u16 of (idx + 1) for a chunk of one batch element.
    xyz_b: [N, 3] f32; q: [m, 3] f32; qn: [m] f32 |q|^2; q64: [m, 3] f64.
    Returns [m, K] int32.

    Re-ranks with bit-exact f32 reference arithmetic (XLA-CPU-matching
    rounding: fma emulated via f64 products). Sort key packs the monotone
    uint32 image of the f32 distance above the 14-bit index, so one uint64
    sort yields (dist asc, idx asc) — the reference's top_k tie order.
    """
    idx = cand_u16.astype(np.int32) - 1
    np.clip(idx, 0, N - 1, out=idx)
    x = xyz_b[idx.reshape(-1)].reshape(*idx.shape, 3)

    x64 = x.astype(np.float64)
    acc = (q64[:, None, 0] * x64[..., 0]).astype(np.float32)
    acc = (q64[:, None, 1] * x64[..., 1] + acc.astype(np.float64)).astype(np.float32)
    acc = (q64[:, None, 2] * x64[..., 2] + acc.astype(np.float64)).astype(np.float32)
    xn = ((x[..., 0] * x[..., 0] + x[..., 1] * x[..., 1]) + x[..., 2] * x[..., 2])
    d = ((np.float32(-2.0) * acc) + qn[:, None]).astype(np.float32) + xn
    d = np.ascontiguousarray(d, dtype=np.float32)

    bits = d.view(np.uint32)
    key = np.where(bits & np.uint32(0x80000000),
                   np.invert(bits), bits | np.uint32(0x80000000))
    comb = (key.astype(np.uint64) << np.uint64(14)) | idx.astype(np.uint64)
    comb.sort(axis=-1)
    return (comb[..., :K] & np.uint64(0x3FFF)).astype(np.int32)


def kernel(xyz, new_xyz):
    xyz = np.ascontiguousarray(xyz, dtype=np.float32)
    new_xyz = np.ascontiguousarray(new_xyz, dtype=np.float32)
    st = _get_state()
    out, = st.sharded(_prep_qa3(xyz, new_xyz), st.next_zeros)
    # rebuild the donated buffer for the next call while this one's
    # transfers are in flight
    st.next_zeros = st.zeros_jit()
    # candidate-independent rerank inputs, computed while transfers fly
    qn = ((new_xyz[..., 0] * new_xyz[..., 0] + new_xyz[..., 1] * new_xyz[..., 1])
          + new_xyz[..., 2] * new_xyz[..., 2])            # [B, M] f32
    q64 = new_xyz.astype(np.float64)
    # Pipeline the D2H with the rerank: each shard's fetch runs concurrently
    # in a worker thread (network wait releases the GIL); the per-batch rerank
    # happens in the same worker as its shard lands (in half-shard chunks for
    # finer GIL interleaving), hiding rerank compute behind the transfers.
    shards = sorted(out.addressable_shards, key=lambda s: s.index[0].start)
    res = np.empty((B, M, K), np.int32)
    H = M // 2

    def work(b):
        cand = np.asarray(shards[b].data)
        for s in (slice(0, H), slice(H, M)):
            res[b, s] = _rerank_one(cand[s], xyz[b], new_xyz[b][s],
                                    qn[b][s], q64[b][s])

    list(st.pool.map(work, range(B)))
    return res


# revision 16
# speedup vs baseline: 1.0996x; 1.0996x over previous
"""KNN top-32 kernel for Trainium2 (Bass/Tile), 8 NeuronCores.

Strategy:
  - Data-parallel over batch: core b handles batch element b (M=4096 queries,
    N=16384 database points, C=3).
  - Host ships raw coords only: qa3 = [2*q | x] as [3, M+N] f32 per core
    (1.97 MB total H2D). No augmented row is materialized: per query tile the
    PE computes s = 2*q.x - |x|^2 (a monotone-decreasing transform of the
    squared distance, per query row) as TWO accumulating K=3 fp32 matmuls
    into one PSUM tile: lhsT=(2q) (3 x 128) @ x (3 x 512), then
    lhsT=(-1s) (3 x 128) @ (x*x) (3 x 512), which adds -|x_n|^2 to every
    query row.
  - DVE reduces each 512-chunk with max8 (top-8 values) + max_index (their
    in-chunk indices) straight out of PSUM into a 256-wide table per 128-query
    tile. The true top-32 of a row is contained in the per-segment top-8 table
    (any segment holds at most 8 of a row's top-32 with overwhelming
    probability for randn inputs).
  - NROUNDS rounds of max8+match_replace(-BIG) on the table mark the top
    NCAND table slots; a compare+multiply turns the paired index table into a
    sparse key array (global_idx+1 at winners, 0 elsewhere), and NROUNDS more
    max8+match_replace rounds compact the candidate indices out, order-free.
    Output as uint16 (global_idx+1 fits 16 bits) to halve the D2H fetch.
  - Host re-ranks the candidates per query with bit-exact f32 reference
    arithmetic (single uint64 sort key: monotone f32 bits << 14 | idx) and
    emits the top-32 indices (int32).

Perf notes (axon-tunneled cores; wall-clock is network-dominated):
  - The jitted shard_map executable is built ONCE and cached; per-call cost is
    H2D of qa3, the NEFF execute (~ms), D2H of the u16 candidates.
  - Per-shard D2H fetches run in worker threads concurrently, and each batch
    element's rerank runs as its shard lands, hiding rerank behind transfers.
  - The per-segment offset table is generated on device with iota (no H2D).
  - The donated output buffers are created on device (no zeros upload).
"""

import numpy as np
from concurrent.futures import ThreadPoolExecutor

import jax
import jax.numpy as jnp
from jax.sharding import Mesh, NamedSharding, PartitionSpec
from jax.experimental.shard_map import shard_map

from concourse import bacc, bass2jax
import concourse.mybir as mybir
from concourse.tile import TileContext

B = 8
M = 4096          # queries per core
N = 16384         # database points per core
K = 32            # neighbors wanted
NROUNDS = 4
NCAND = 8 * NROUNDS  # candidates extracted per query
SEG = 512
G = N // SEG      # 32 segments -> table width 256
TW = G * 8        # table width
MT = 128          # query rows per tile
NT = M // MT      # 32 row tiles
NEG = -1.0e30

F32 = mybir.dt.float32
I32 = mybir.dt.int32
U16 = mybir.dt.uint16


def build_bass():
    nc = bacc.Bacc()
    qa3 = nc.declare_dram_parameter("qa3", [3, M + N], F32, isOutput=False)
    out = nc.declare_dram_parameter("out", [M, NCAND], U16, isOutput=True)

    with TileContext(nc) as tc, \
         tc.tile_pool(name="const", bufs=1) as cpool, \
         tc.tile_pool(name="work", bufs=2) as wpool, \
         tc.tile_pool(name="outp", bufs=3) as opool, \
         tc.tile_pool(name="psum", bufs=8, space="PSUM") as ppool:
        qx = cpool.tile([3, M + N], F32)
        nc.sync.dma_start(out=qx[:, :], in_=qa3[:, :])
        qs = qx[:, :M]
        asb = qx[:, M:]
        # database squares (for the -|x|^2 accumulation matmul)
        sq = cpool.tile([3, N], F32)
        nc.vector.tensor_mul(sq[:, :], asb[:, :], asb[:, :])
        # [3, MT] of -1: second matmul adds Sum_c -sq[c,n] = -|x_n|^2 to every
        # query row of the PSUM tile
        neg1 = cpool.tile([3, MT], F32)
        nc.vector.memset(neg1[:, :], -1.0)

        # per-slot global offset table: slot j -> (j//8)*SEG + 1 (same for all
        # partitions), generated on device instead of shipped over the tunnel
        ioff = cpool.tile([MT, TW], I32)
        nc.gpsimd.iota(ioff[:, :], [[SEG, G], [0, 8]], base=1,
                       channel_multiplier=0)
        offt = cpool.tile([MT, TW], F32)
        nc.vector.tensor_copy(offt[:, :], ioff[:, :])

        for t in range(NT):
            tbl = wpool.tile([MT, TW], F32, tag="tbl")
            idx16 = wpool.tile([MT, TW], U16, tag="idx16")
            for g in range(G):
                ps = ppool.tile([MT, SEG], F32, tag="ps")
                nc.tensor.matmul(
                    ps[:, :],
                    qs[:, t * MT:(t + 1) * MT],
                    asb[:, g * SEG:(g + 1) * SEG],
                    start=True,
                    stop=False,
                )
                nc.tensor.matmul(
                    ps[:, :],
                    neg1[:, :],
                    sq[:, g * SEG:(g + 1) * SEG],
                    start=False,
                    stop=True,
                )
                nc.vector.max(out=tbl[:, g * 8:(g + 1) * 8], in_=ps[:, :])
                nc.vector.max_index(
                    out=idx16[:, g * 8:(g + 1) * 8],
                    in_max=tbl[:, g * 8:(g + 1) * 8],
                    in_values=ps[:, :],
                )
            # paired global index table (value = global idx + 1) as f32
            idxf = wpool.tile([MT, TW], F32, tag="idxf")
            nc.vector.tensor_copy(idxf[:, :], idx16[:, :])
            nc.vector.tensor_add(idxf[:, :], idxf[:, :], offt[:, :])
            # pop top-NCAND values; winners' slots become NEG
            v8 = wpool.tile([MT, 8], F32, tag="v8")
            for r in range(NROUNDS):
                nc.vector.max(out=v8[:, :], in_=tbl[:, :])
                nc.vector.match_replace(
                    out=tbl[:, :], in_to_replace=v8[:, :], in_values=tbl[:, :],
                    imm_value=NEG,
                )
            # sparse key array: idx+1 where popped, 0 elsewhere
            wmask = wpool.tile([MT, TW], F32, tag="wmask")
            nc.vector.tensor_scalar(
                wmask[:, :], tbl[:, :], NEG, None, op0=mybir.AluOpType.is_equal
            )
            key = wpool.tile([MT, TW], F32, tag="key")
            nc.vector.tensor_mul(key[:, :], wmask[:, :], idxf[:, :])
            # compact the NCAND winning indices (order-free)
            outt = opool.tile([MT, NCAND], F32, tag="outt")
            for r in range(NROUNDS):
                nc.vector.max(out=outt[:, r * 8:(r + 1) * 8], in_=key[:, :])
                if r < NROUNDS - 1:
                    nc.vector.match_replace(
                        out=key[:, :], in_to_replace=outt[:, r * 8:(r + 1) * 8],
                        in_values=key[:, :], imm_value=0.0,
                    )
            out16 = opool.tile([MT, NCAND], U16, tag="out16")
            nc.vector.tensor_copy(out16[:, :], outt[:, :])
            nc.sync.dma_start(out=out[t * MT:(t + 1) * MT, :], in_=out16[:, :])
    nc.finalize()
    return nc


class _State:
    __slots__ = ("nc", "sharded", "zeros_jit", "pool", "next_zeros")


_STATE = None


def _get_state():
    global _STATE
    if _STATE is not None:
        return _STATE
    nc = build_bass()
    bass2jax.install_neuronx_cc_hook()

    partition_name = nc.partition_id_tensor.name if nc.partition_id_tensor else None
    in_names, out_names, out_avals = [], [], []
    for alloc in nc.m.functions[0].allocations:
        if not isinstance(alloc, mybir.MemoryLocationSet):
            continue
        name = alloc.memorylocations[0].name
        if alloc.kind == "ExternalInput":
            if name != partition_name:
                in_names.append(name)
        elif alloc.kind == "ExternalOutput":
            out_names.append(name)
            out_avals.append(jax.core.ShapedArray(
                tuple(alloc.tensor_shape), mybir.dt.np(alloc.dtype)))
    assert in_names == ["qa3"] and out_names == ["out"], (in_names, out_names)
    all_names = list(in_names + out_names)
    if partition_name is not None:
        all_names.append(partition_name)
    n_params = len(in_names)

    def _body(*args):
        operands = list(args)
        if partition_name is not None:
            operands.append(bass2jax.partition_id_tensor())
        outs = bass2jax._bass_exec_p.bind(
            *operands,
            out_avals=tuple(out_avals),
            in_names=tuple(all_names),
            out_names=tuple(out_names),
            lowering_input_output_aliases=(),
            sim_require_finite=True,
            sim_require_nnan=True,
            nc=nc,
        )
        return tuple(outs)

    devices = jax.devices()[:B]
    assert len(devices) == B, f"need {B} devices, got {len(jax.devices())}"
    mesh = Mesh(np.asarray(devices), ("core",))
    spec = PartitionSpec("core")
    sharded = jax.jit(
        shard_map(_body, mesh=mesh, in_specs=(spec,) * (n_params + 1),
                  out_specs=(spec,), check_rep=False),
        donate_argnums=(n_params,),
        keep_unused=True,
    )
    zeros_jit = jax.jit(
        lambda: jnp.zeros((B * M, NCAND), jnp.uint16),
        out_shardings=NamedSharding(mesh, spec),
    )
    st = _State()
    st.nc = nc
    st.sharded = sharded
    st.zeros_jit = zeros_jit
    st.pool = ThreadPoolExecutor(B)
    st.next_zeros = zeros_jit()
    _STATE = st
    return st


def _prep_qa3(xyz, new_xyz):
    """Concatenated per-core raw-coord matrices: [B*3, M+N] f32.

    Row block b: [2*qx, 2*qy, 2*qz | x, y, z] for batch element b. The query
    side carries the factor 2 (exact in fp32) so the device matmul computes
    s = 2*q.x - |x|^2 with the same rounding as scaling the database side.
    """
    qa3 = np.empty((B, 3, M + N), np.float32)
    np.multiply(new_xyz.transpose(0, 2, 1), np.float32(2.0), out=qa3[:, :, :M])
    qa3[:, :, M:] = xyz.transpose(0, 2, 1)
    return qa3.reshape(B * 3, M + N)


def _rerank_one(cand_u16, xyz_b, q, qn, q64):
    """cand_u16: [m, NCAND] u16 of (idx + 1) for a chunk of one batch element.
    xyz_b: [N, 3] f32; q: [m, 3] f32; qn: [m] f32 |q|^2; q64: [m, 3] f64.
    Returns [m, K] int32.

    Re-ranks with bit-exact f32 reference arithmetic (XLA-CPU-matching
    rounding: fma emulated via f64 products). Sort key packs the monotone
    uint32 image of the f32 distance above the 14-bit index, so one uint64
    sort yields (dist asc, idx asc) — the reference's top_k tie order.
    """
    idx = cand_u16.astype(np.int32) - 1
    np.clip(idx, 0, N - 1, out=idx)
    x = xyz_b[idx.reshape(-1)].reshape(*idx.shape, 3)

    x64 = x.astype(np.float64)
    acc = (q64[:, None, 0] * x64[..., 0]).astype(np.float32)
    acc = (q64[:, None, 1] * x64[..., 1] + acc.astype(np.float64)).astype(np.float32)
    acc = (q64[:, None, 2] * x64[..., 2] + acc.astype(np.float64)).astype(np.float32)
    xn = ((x[..., 0] * x[..., 0] + x[..., 1] * x[..., 1]) + x[..., 2] * x[..., 2])
    d = ((np.float32(-2.0) * acc) + qn[:, None]).astype(np.float32) + xn
    d = np.ascontiguousarray(d, dtype=np.float32)

    bits = d.view(np.uint32)
    key = np.where(bits & np.uint32(0x80000000),
                   np.invert(bits), bits | np.uint32(0x80000000))
    comb = (key.astype(np.uint64) << np.uint64(14)) | idx.astype(np.uint64)
    comb.sort(axis=-1)
    return (comb[..., :K] & np.uint64(0x3FFF)).astype(np.int32)


def kernel(xyz, new_xyz):
    xyz = np.ascontiguousarray(xyz, dtype=np.float32)
    new_xyz = np.ascontiguousarray(new_xyz, dtype=np.float32)
    st = _get_state()
    out, = st.sharded(_prep_qa3(xyz, new_xyz), st.next_zeros)
    # rebuild the donated buffer for the next call while this one's
    # transfers are in flight
    st.next_zeros = st.zeros_jit()
    # candidate-independent rerank inputs, computed while transfers fly
    qn = ((new_xyz[..., 0] * new_xyz[..., 0] + new_xyz[..., 1] * new_xyz[..., 1])
          + new_xyz[..., 2] * new_xyz[..., 2])            # [B, M] f32
    q64 = new_xyz.astype(np.float64)
    # Pipeline the D2H with the rerank: each shard's fetch runs concurrently
    # in a worker thread (network wait releases the GIL); the per-batch rerank
    # happens in the same worker as its shard lands (in half-shard chunks for
    # finer GIL interleaving), hiding rerank compute behind the transfers.
    shards = sorted(out.addressable_shards, key=lambda s: s.index[0].start)
    res = np.empty((B, M, K), np.int32)
    H = M // 2

    def work(b):
        cand = np.asarray(shards[b].data)
        for s in (slice(0, H), slice(H, M)):
            res[b, s] = _rerank_one(cand[s], xyz[b], new_xyz[b][s],
                                    qn[b][s], q64[b][s])

    list(st.pool.map(work, range(B)))
    return res


# revision 17
# speedup vs baseline: 1.1271x; 1.0250x over previous
"""KNN top-32 kernel for Trainium2 (Bass/Tile), 8 NeuronCores.

Strategy:
  - Data-parallel over batch: core b handles batch element b (M=4096 queries,
    N=16384 database points, C=3).
  - Host ships raw coords only: qa3 = [2*q | x] as [3, M+N] f32 per core
    (1.97 MB total H2D). No augmented row is materialized: per query tile the
    PE computes s = 2*q.x - |x|^2 (a monotone-decreasing transform of the
    squared distance, per query row) as TWO accumulating K=3 fp32 matmuls
    into one PSUM tile: lhsT=(2q) (3 x 128) @ x (3 x 512), then
    lhsT=(-1s) (3 x 128) @ (x*x) (3 x 512), which adds -|x_n|^2 to every
    query row.
  - DVE reduces each 512-chunk with max8 (top-8 values) + max_index (their
    in-chunk indices) straight out of PSUM into a 256-wide table per 128-query
    tile. The true top-32 of a row is contained in the per-segment top-8 table
    (any segment holds at most 8 of a row's top-32 with overwhelming
    probability for randn inputs).
  - NROUNDS rounds of max8+match_replace(-BIG) on the table mark the top
    NCAND table slots; a compare+multiply turns the paired index table into a
    sparse key array (global_idx+1 at winners, 0 elsewhere), and NROUNDS more
    max8+match_replace rounds compact the candidate indices out, order-free.
    Output as uint16 (global_idx+1 fits 16 bits) to halve the D2H fetch.
  - Host re-ranks the candidates per query with bit-exact f32 reference
    arithmetic (single uint64 sort key: monotone f32 bits << 14 | idx) and
    emits the top-32 indices (int32).

Perf notes (axon-tunneled cores; wall-clock is network-dominated):
  - The jitted shard_map executable is built ONCE and cached; per-call cost is
    H2D of qa3, the NEFF execute (~ms), D2H of the u16 candidates.
  - Per-shard D2H fetches run in worker threads concurrently, and each batch
    element's rerank runs as its shard lands, hiding rerank behind transfers.
  - The per-segment offset table is generated on device with iota (no H2D).
  - The donated output buffers are created on device (no zeros upload).
"""

import numpy as np
from concurrent.futures import ThreadPoolExecutor

import jax
import jax.numpy as jnp
from jax.sharding import Mesh, NamedSharding, PartitionSpec
from jax.experimental.shard_map import shard_map

from concourse import bacc, bass2jax
import concourse.mybir as mybir
from concourse.tile import TileContext

B = 8
M = 4096          # queries per core
N = 16384         # database points per core
K = 32            # neighbors wanted
NROUNDS = 4
NCAND = 8 * NROUNDS  # candidates extracted per query
SEG = 512
G = N // SEG      # 32 segments -> table width 256
TW = G * 8        # table width
MT = 128          # query rows per tile
NT = M // MT      # 32 row tiles
NEG = -1.0e30

F32 = mybir.dt.float32
I32 = mybir.dt.int32
U16 = mybir.dt.uint16


def build_bass():
    nc = bacc.Bacc()
    qa3 = nc.declare_dram_parameter("qa3", [3, M + N], F32, isOutput=False)
    out = nc.declare_dram_parameter("out", [M, NCAND], U16, isOutput=True)

    with TileContext(nc) as tc, \
         tc.tile_pool(name="const", bufs=1) as cpool, \
         tc.tile_pool(name="work", bufs=2) as wpool, \
         tc.tile_pool(name="outp", bufs=3) as opool, \
         tc.tile_pool(name="psum", bufs=8, space="PSUM") as ppool:
        qx = cpool.tile([3, M + N], F32)
        nc.sync.dma_start(out=qx[:, :], in_=qa3[:, :])
        qs = qx[:, :M]
        asb = qx[:, M:]
        # database squares (for the -|x|^2 accumulation matmul)
        sq = cpool.tile([3, N], F32)
        nc.vector.tensor_mul(sq[:, :], asb[:, :], asb[:, :])
        # [3, MT] of -1: second matmul adds Sum_c -sq[c,n] = -|x_n|^2 to every
        # query row of the PSUM tile
        neg1 = cpool.tile([3, MT], F32)
        nc.vector.memset(neg1[:, :], -1.0)

        # per-slot global offset table: slot j -> (j//8)*SEG + 1 (same for all
        # partitions), generated on device instead of shipped over the tunnel
        ioff = cpool.tile([MT, TW], I32)
        nc.gpsimd.iota(ioff[:, :], [[SEG, G], [0, 8]], base=1,
                       channel_multiplier=0)
        offt = cpool.tile([MT, TW], F32)
        nc.vector.tensor_copy(offt[:, :], ioff[:, :])

        for t in range(NT):
            tbl = wpool.tile([MT, TW], F32, tag="tbl")
            idx16 = wpool.tile([MT, TW], U16, tag="idx16")
            for g in range(G):
                ps = ppool.tile([MT, SEG], F32, tag="ps")
                nc.tensor.matmul(
                    ps[:, :],
                    qs[:, t * MT:(t + 1) * MT],
                    asb[:, g * SEG:(g + 1) * SEG],
                    start=True,
                    stop=False,
                )
                nc.tensor.matmul(
                    ps[:, :],
                    neg1[:, :],
                    sq[:, g * SEG:(g + 1) * SEG],
                    start=False,
                    stop=True,
                )
                nc.vector.max(out=tbl[:, g * 8:(g + 1) * 8], in_=ps[:, :])
                nc.vector.max_index(
                    out=idx16[:, g * 8:(g + 1) * 8],
                    in_max=tbl[:, g * 8:(g + 1) * 8],
                    in_values=ps[:, :],
                )
            # paired global index table (value = global idx + 1) as f32
            idxf = wpool.tile([MT, TW], F32, tag="idxf")
            nc.vector.tensor_copy(idxf[:, :], idx16[:, :])
            nc.vector.tensor_add(idxf[:, :], idxf[:, :], offt[:, :])
            # pop top-NCAND values; winners' slots become NEG
            v8 = wpool.tile([MT, 8], F32, tag="v8")
            for r in range(NROUNDS):
                nc.vector.max(out=v8[:, :], in_=tbl[:, :])
                nc.vector.match_replace(
                    out=tbl[:, :], in_to_replace=v8[:, :], in_values=tbl[:, :],
                    imm_value=NEG,
                )
            # sparse key array: idx+1 where popped, 0 elsewhere
            wmask = wpool.tile([MT, TW], F32, tag="wmask")
            nc.vector.tensor_scalar(
                wmask[:, :], tbl[:, :], NEG, None, op0=mybir.AluOpType.is_equal
            )
            key = wpool.tile([MT, TW], F32, tag="key")
            nc.vector.tensor_mul(key[:, :], wmask[:, :], idxf[:, :])
            # compact the NCAND winning indices (order-free)
            outt = opool.tile([MT, NCAND], F32, tag="outt")
            for r in range(NROUNDS):
                nc.vector.max(out=outt[:, r * 8:(r + 1) * 8], in_=key[:, :])
                if r < NROUNDS - 1:
                    nc.vector.match_replace(
                        out=key[:, :], in_to_replace=outt[:, r * 8:(r + 1) * 8],
                        in_values=key[:, :], imm_value=0.0,
                    )
            out16 = opool.tile([MT, NCAND], U16, tag="out16")
            nc.vector.tensor_copy(out16[:, :], outt[:, :])
            nc.sync.dma_start(out=out[t * MT:(t + 1) * MT, :], in_=out16[:, :])
    nc.finalize()
    return nc


class _State:
    __slots__ = ("nc", "sharded", "zeros_jit", "pool", "next_zeros")


_STATE = None


def _get_state():
    global _STATE
    if _STATE is not None:
        return _STATE
    nc = build_bass()
    bass2jax.install_neuronx_cc_hook()

    partition_name = nc.partition_id_tensor.name if nc.partition_id_tensor else None
    in_names, out_names, out_avals = [], [], []
    for alloc in nc.m.functions[0].allocations:
        if not isinstance(alloc, mybir.MemoryLocationSet):
            continue
        name = alloc.memorylocations[0].name
        if alloc.kind == "ExternalInput":
            if name != partition_name:
                in_names.append(name)
        elif alloc.kind == "ExternalOutput":
            out_names.append(name)
            out_avals.append(jax.core.ShapedArray(
                tuple(alloc.tensor_shape), mybir.dt.np(alloc.dtype)))
    assert in_names == ["qa3"] and out_names == ["out"], (in_names, out_names)
    all_names = list(in_names + out_names)
    if partition_name is not None:
        all_names.append(partition_name)
    n_params = len(in_names)

    def _body(*args):
        operands = list(args)
        if partition_name is not None:
            operands.append(bass2jax.partition_id_tensor())
        outs = bass2jax._bass_exec_p.bind(
            *operands,
            out_avals=tuple(out_avals),
            in_names=tuple(all_names),
            out_names=tuple(out_names),
            lowering_input_output_aliases=(),
            sim_require_finite=True,
            sim_require_nnan=True,
            nc=nc,
        )
        return tuple(outs)

    devices = jax.devices()[:B]
    assert len(devices) == B, f"need {B} devices, got {len(jax.devices())}"
    mesh = Mesh(np.asarray(devices), ("core",))
    spec = PartitionSpec("core")
    sharded = jax.jit(
        shard_map(_body, mesh=mesh, in_specs=(spec,) * (n_params + 1),
                  out_specs=(spec,), check_rep=False),
        donate_argnums=(n_params,),
        keep_unused=True,
    )
    zeros_jit = jax.jit(
        lambda: jnp.zeros((B * M, NCAND), jnp.uint16),
        out_shardings=NamedSharding(mesh, spec),
    )
    st = _State()
    st.nc = nc
    st.sharded = sharded
    st.zeros_jit = zeros_jit
    st.pool = ThreadPoolExecutor(B)
    st.next_zeros = zeros_jit()
    _STATE = st
    return st


def _prep_qa3(xyz, new_xyz):
    """Concatenated per-core raw-coord matrices: [B*3, M+N] f32.

    Row block b: [2*qx, 2*qy, 2*qz | x, y, z] for batch element b. The query
    side carries the factor 2 (exact in fp32) so the device matmul computes
    s = 2*q.x - |x|^2 with the same rounding as scaling the database side.
    """
    qa3 = np.empty((B, 3, M + N), np.float32)
    np.multiply(new_xyz.transpose(0, 2, 1), np.float32(2.0), out=qa3[:, :, :M])
    qa3[:, :, M:] = xyz.transpose(0, 2, 1)
    return qa3.reshape(B * 3, M + N)


def _rerank_one(cand_u16, xyz_b, q, qn, q64):
    """cand_u16: [m, NCAND] u16 of (idx + 1) for a chunk of one batch element.
    xyz_b: [N, 3] f32; q: [m, 3] f32; qn: [m] f32 |q|^2; q64: [m, 3] f64.
    Returns [m, K] int32.

    Re-ranks with bit-exact f32 reference arithmetic (XLA-CPU-matching
    rounding: fma emulated via f64 products). Sort key packs the monotone
    uint32 image of the f32 distance above the 14-bit index, so one uint64
    sort yields (dist asc, idx asc) — the reference's top_k tie order.
    """
    idx = cand_u16.astype(np.int32) - 1
    np.clip(idx, 0, N - 1, out=idx)
    x = xyz_b[idx.reshape(-1)].reshape(*idx.shape, 3)

    x64 = x.astype(np.float64)
    acc = (q64[:, None, 0] * x64[..., 0]).astype(np.float32)
    acc = (q64[:, None, 1] * x64[..., 1] + acc.astype(np.float64)).astype(np.float32)
    acc = (q64[:, None, 2] * x64[..., 2] + acc.astype(np.float64)).astype(np.float32)
    xn = ((x[..., 0] * x[..., 0] + x[..., 1] * x[..., 1]) + x[..., 2] * x[..., 2])
    d = ((np.float32(-2.0) * acc) + qn[:, None]).astype(np.float32) + xn
    d = np.ascontiguousarray(d, dtype=np.float32)

    bits = d.view(np.uint32)
    key = np.where(bits & np.uint32(0x80000000),
                   np.invert(bits), bits | np.uint32(0x80000000))
    comb = (key.astype(np.uint64) << np.uint64(14)) | idx.astype(np.uint64)
    comb.sort(axis=-1)
    return (comb[..., :K] & np.uint64(0x3FFF)).astype(np.int32)


def kernel(xyz, new_xyz):
    xyz = np.ascontiguousarray(xyz, dtype=np.float32)
    new_xyz = np.ascontiguousarray(new_xyz, dtype=np.float32)
    st = _get_state()
    qa3 = _prep_qa3(xyz, new_xyz)
    try:
        out, = st.sharded(qa3, st.next_zeros)
    except Exception:
        # the donated buffer may be gone after a failed dispatch — retry once
        st.next_zeros = st.zeros_jit()
        out, = st.sharded(qa3, st.next_zeros)
    # rebuild the donated buffer for the next call while this one's
    # transfers are in flight
    st.next_zeros = st.zeros_jit()
    # candidate-independent rerank inputs, computed while transfers fly
    qn = ((new_xyz[..., 0] * new_xyz[..., 0] + new_xyz[..., 1] * new_xyz[..., 1])
          + new_xyz[..., 2] * new_xyz[..., 2])            # [B, M] f32
    q64 = new_xyz.astype(np.float64)
    # Pipeline the D2H with the rerank: each shard's fetch runs concurrently
    # in a worker thread (network wait releases the GIL); the per-batch rerank
    # happens in the same worker as its shard lands (in half-shard chunks for
    # finer GIL interleaving), hiding rerank compute behind the transfers.
    shards = sorted(out.addressable_shards, key=lambda s: s.index[0].start)
    res = np.empty((B, M, K), np.int32)
    H = M // 2

    def work(b):
        cand = np.asarray(shards[b].data)
        for s in (slice(0, H), slice(H, M)):
            res[b, s] = _rerank_one(cand[s], xyz[b], new_xyz[b][s],
                                    qn[b][s], q64[b][s])

    list(st.pool.map(work, range(B)))
    return res


# revision 18
# speedup vs baseline: 1.2039x; 1.0681x over previous
"""KNN top-32 kernel for Trainium2 (Bass/Tile), 8 NeuronCores.

Strategy:
  - Data-parallel over batch: core b handles batch element b (M=4096 queries,
    N=16384 database points, C=3).
  - Host ships raw coords only: qa3 = [2*q | x] as [3, M+N] f32 per core
    (1.97 MB total H2D). No augmented row is materialized: per query tile the
    PE computes s = 2*q.x - |x|^2 (a monotone-decreasing transform of the
    squared distance, per query row) as TWO accumulating K=3 fp32 matmuls
    into one PSUM tile: lhsT=(2q) (3 x 128) @ x (3 x 512), then
    lhsT=(-1s) (3 x 128) @ (x*x) (3 x 512), which adds -|x_n|^2 to every
    query row.
  - DVE reduces each 512-chunk with max8 (top-8 values) + max_index (their
    in-chunk indices) straight out of PSUM into a 256-wide table per 128-query
    tile. The true top-32 of a row is contained in the per-segment top-8 table
    (any segment holds at most 8 of a row's top-32 with overwhelming
    probability for randn inputs).
  - NROUNDS rounds of max8+match_replace(-BIG) on the table mark the top
    NCAND table slots; a compare+multiply turns the paired index table into a
    sparse key array (global_idx+1 at winners, 0 elsewhere), and NROUNDS more
    max8+match_replace rounds compact the candidate indices out, order-free.
    Output as uint16 (global_idx+1 fits 16 bits) to halve the D2H fetch.
  - Host re-ranks the candidates per query with bit-exact f32 reference
    arithmetic (single uint64 sort key: monotone f32 bits << 14 | idx) and
    emits the top-32 indices (int32).

Perf notes (axon-tunneled cores; wall-clock is network-dominated):
  - The jitted shard_map executable is built ONCE and cached; per-call cost is
    H2D of qa3, the NEFF execute (~ms), D2H of the u16 candidates.
  - Per-shard D2H fetches run in worker threads concurrently, and each batch
    element's rerank runs as its shard lands, hiding rerank behind transfers.
  - The per-segment offset table is generated on device with iota (no H2D).
  - The donated output buffers are created on device (no zeros upload).
"""

import numpy as np
from concurrent.futures import ThreadPoolExecutor

import jax
import jax.numpy as jnp
from jax.sharding import Mesh, NamedSharding, PartitionSpec
from jax.experimental.shard_map import shard_map

from concourse import bacc, bass2jax
import concourse.mybir as mybir
from concourse.tile import TileContext

B = 8
M = 4096          # queries per core
N = 16384         # database points per core
K = 32            # neighbors wanted
NROUNDS = 4
NCAND = 8 * NROUNDS  # candidates extracted per query
SEG = 512
G = N // SEG      # 32 segments -> table width 256
TW = G * 8        # table width
MT = 128          # query rows per tile
NT = M // MT      # 32 row tiles
NEG = -1.0e30

F32 = mybir.dt.float32
I32 = mybir.dt.int32
U16 = mybir.dt.uint16


def build_bass():
    nc = bacc.Bacc()
    qa3 = nc.declare_dram_parameter("qa3", [3, M + N], F32, isOutput=False)
    out = nc.declare_dram_parameter("out", [M, NCAND], U16, isOutput=True)

    with TileContext(nc) as tc, \
         tc.tile_pool(name="const", bufs=1) as cpool, \
         tc.tile_pool(name="work", bufs=2) as wpool, \
         tc.tile_pool(name="outp", bufs=3) as opool, \
         tc.tile_pool(name="psum", bufs=8, space="PSUM") as ppool:
        qx = cpool.tile([3, M + N], F32)
        nc.sync.dma_start(out=qx[:, :], in_=qa3[:, :])
        qs = qx[:, :M]
        asb = qx[:, M:]
        # database squares (for the -|x|^2 accumulation matmul)
        sq = cpool.tile([3, N], F32)
        nc.vector.tensor_mul(sq[:, :], asb[:, :], asb[:, :])
        # [3, MT] of -1: second matmul adds Sum_c -sq[c,n] = -|x_n|^2 to every
        # query row of the PSUM tile
        neg1 = cpool.tile([3, MT], F32)
        nc.vector.memset(neg1[:, :], -1.0)

        # per-slot global offset table: slot j -> (j//8)*SEG + 1 (same for all
        # partitions), generated on device instead of shipped over the tunnel
        ioff = cpool.tile([MT, TW], I32)
        nc.gpsimd.iota(ioff[:, :], [[SEG, G], [0, 8]], base=1,
                       channel_multiplier=0)
        offt = cpool.tile([MT, TW], F32)
        nc.vector.tensor_copy(offt[:, :], ioff[:, :])

        for t in range(NT):
            tbl = wpool.tile([MT, TW], F32, tag="tbl")
            idx16 = wpool.tile([MT, TW], U16, tag="idx16")
            for g in range(G):
                ps = ppool.tile([MT, SEG], F32, tag="ps")
                nc.tensor.matmul(
                    ps[:, :],
                    qs[:, t * MT:(t + 1) * MT],
                    asb[:, g * SEG:(g + 1) * SEG],
                    start=True,
                    stop=False,
                )
                nc.tensor.matmul(
                    ps[:, :],
                    neg1[:, :],
                    sq[:, g * SEG:(g + 1) * SEG],
                    start=False,
                    stop=True,
                )
                nc.vector.max(out=tbl[:, g * 8:(g + 1) * 8], in_=ps[:, :])
                nc.vector.max_index(
                    out=idx16[:, g * 8:(g + 1) * 8],
                    in_max=tbl[:, g * 8:(g + 1) * 8],
                    in_values=ps[:, :],
                )
            # paired global index table (value = global idx + 1) as f32
            idxf = wpool.tile([MT, TW], F32, tag="idxf")
            nc.vector.tensor_copy(idxf[:, :], idx16[:, :])
            nc.vector.tensor_add(idxf[:, :], idxf[:, :], offt[:, :])
            # pop top-NCAND values; winners' slots become NEG
            v8 = wpool.tile([MT, 8], F32, tag="v8")
            for r in range(NROUNDS):
                nc.vector.max(out=v8[:, :], in_=tbl[:, :])
                nc.vector.match_replace(
                    out=tbl[:, :], in_to_replace=v8[:, :], in_values=tbl[:, :],
                    imm_value=NEG,
                )
            # sparse key array: idx+1 where popped, 0 elsewhere
            wmask = wpool.tile([MT, TW], F32, tag="wmask")
            nc.vector.tensor_scalar(
                wmask[:, :], tbl[:, :], NEG, None, op0=mybir.AluOpType.is_equal
            )
            key = wpool.tile([MT, TW], F32, tag="key")
            nc.vector.tensor_mul(key[:, :], wmask[:, :], idxf[:, :])
            # compact the NCAND winning indices (order-free)
            outt = opool.tile([MT, NCAND], F32, tag="outt")
            for r in range(NROUNDS):
                nc.vector.max(out=outt[:, r * 8:(r + 1) * 8], in_=key[:, :])
                if r < NROUNDS - 1:
                    nc.vector.match_replace(
                        out=key[:, :], in_to_replace=outt[:, r * 8:(r + 1) * 8],
                        in_values=key[:, :], imm_value=0.0,
                    )
            out16 = opool.tile([MT, NCAND], U16, tag="out16")
            nc.vector.tensor_copy(out16[:, :], outt[:, :])
            nc.sync.dma_start(out=out[t * MT:(t + 1) * MT, :], in_=out16[:, :])
    nc.finalize()
    return nc


class _State:
    __slots__ = ("nc", "sharded", "zeros_jit", "pool", "next_zeros")


_STATE = None


def _get_state():
    global _STATE
    if _STATE is not None:
        return _STATE
    nc = build_bass()
    bass2jax.install_neuronx_cc_hook()

    partition_name = nc.partition_id_tensor.name if nc.partition_id_tensor else None
    in_names, out_names, out_avals = [], [], []
    for alloc in nc.m.functions[0].allocations:
        if not isinstance(alloc, mybir.MemoryLocationSet):
            continue
        name = alloc.memorylocations[0].name
        if alloc.kind == "ExternalInput":
            if name != partition_name:
                in_names.append(name)
        elif alloc.kind == "ExternalOutput":
            out_names.append(name)
            out_avals.append(jax.core.ShapedArray(
                tuple(alloc.tensor_shape), mybir.dt.np(alloc.dtype)))
    assert in_names == ["qa3"] and out_names == ["out"], (in_names, out_names)
    all_names = list(in_names + out_names)
    if partition_name is not None:
        all_names.append(partition_name)
    n_params = len(in_names)

    def _body(*args):
        operands = list(args)
        if partition_name is not None:
            operands.append(bass2jax.partition_id_tensor())
        outs = bass2jax._bass_exec_p.bind(
            *operands,
            out_avals=tuple(out_avals),
            in_names=tuple(all_names),
            out_names=tuple(out_names),
            lowering_input_output_aliases=(),
            sim_require_finite=True,
            sim_require_nnan=True,
            nc=nc,
        )
        return tuple(outs)

    devices = jax.devices()[:B]
    assert len(devices) == B, f"need {B} devices, got {len(jax.devices())}"
    mesh = Mesh(np.asarray(devices), ("core",))
    spec = PartitionSpec("core")
    sharded = jax.jit(
        shard_map(_body, mesh=mesh, in_specs=(spec,) * (n_params + 1),
                  out_specs=(spec,), check_rep=False),
        donate_argnums=(n_params,),
        keep_unused=True,
    )
    zeros_jit = jax.jit(
        lambda: jnp.zeros((B * M, NCAND), jnp.uint16),
        out_shardings=NamedSharding(mesh, spec),
    )
    st = _State()
    st.nc = nc
    st.sharded = sharded
    st.zeros_jit = zeros_jit
    st.pool = ThreadPoolExecutor(B)
    st.next_zeros = zeros_jit()
    _STATE = st
    return st


def _prep_qa3(xyz, new_xyz):
    """Concatenated per-core raw-coord matrices: [B*3, M+N] f32.

    Row block b: [2*qx, 2*qy, 2*qz | x, y, z] for batch element b. The query
    side carries the factor 2 (exact in fp32) so the device matmul computes
    s = 2*q.x - |x|^2 with the same rounding as scaling the database side.
    """
    qa3 = np.empty((B, 3, M + N), np.float32)
    np.multiply(new_xyz.transpose(0, 2, 1), np.float32(2.0), out=qa3[:, :, :M])
    qa3[:, :, M:] = xyz.transpose(0, 2, 1)
    return qa3.reshape(B * 3, M + N)


def _rerank_chunk(idx, xyz_b, xn_b, qn_c, q64_c):
    """idx: [m, NCAND] int32 candidate indices (0-based, clipped) for a chunk
    of one batch element. xyz_b: [N, 3] f32; xn_b: [N] f32 |x|^2 table;
    qn_c: [m] f32 |q|^2; q64_c: [m, 3] f64 queries. Returns [m, K] int32.

    Re-ranks with bit-exact f32 reference arithmetic (XLA-CPU-matching
    rounding: fma emulated via f64 products). Sort key packs the monotone
    uint32 image of the f32 distance above the 14-bit index, so one uint64
    sort yields (dist asc, idx asc) — the reference's top_k tie order. When
    every distance in the chunk is >= 0 the raw IEEE bits are already
    monotone (and a sum ending in +xn can never round to -0.0), so the
    sign-flip mapping is skipped.
    """
    flat = idx.reshape(-1)
    x = np.take(xyz_b, flat, axis=0).reshape(*idx.shape, 3)

    x64 = x.astype(np.float64)
    t = np.multiply(q64_c[:, None, 0], x64[..., 0])
    acc = t.astype(np.float32)
    np.multiply(q64_c[:, None, 1], x64[..., 1], out=t)
    t += acc
    acc = t.astype(np.float32)
    np.multiply(q64_c[:, None, 2], x64[..., 2], out=t)
    t += acc
    acc = t.astype(np.float32)
    xn = np.take(xn_b, flat).reshape(idx.shape)
    d = ((np.float32(-2.0) * acc) + qn_c[:, None]).astype(np.float32) + xn
    d = np.ascontiguousarray(d, dtype=np.float32)

    bits = d.view(np.uint32)
    if d.min() >= 0.0:
        key = bits
    else:
        key = np.where(bits & np.uint32(0x80000000),
                       np.invert(bits), bits | np.uint32(0x80000000))
    comb = (key.astype(np.uint64) << np.uint64(14)) | idx.astype(np.uint64)
    comb.sort(axis=-1)
    return (comb[..., :K] & np.uint64(0x3FFF)).astype(np.int32)


def kernel(xyz, new_xyz):
    xyz = np.ascontiguousarray(xyz, dtype=np.float32)
    new_xyz = np.ascontiguousarray(new_xyz, dtype=np.float32)
    st = _get_state()
    qa3 = _prep_qa3(xyz, new_xyz)
    try:
        out, = st.sharded(qa3, st.next_zeros)
    except Exception:
        # the donated buffer may be gone after a failed dispatch — retry once
        st.next_zeros = st.zeros_jit()
        out, = st.sharded(qa3, st.next_zeros)
    # rebuild the donated buffer for the next call while this one's
    # transfers are in flight
    st.next_zeros = st.zeros_jit()
    # candidate-independent rerank inputs, computed during the ~90 ms network
    # wait: query norms/f64 copies and the full database |x|^2 table (gathered
    # per candidate later instead of recomputed from coords)
    qn = ((new_xyz[..., 0] * new_xyz[..., 0] + new_xyz[..., 1] * new_xyz[..., 1])
          + new_xyz[..., 2] * new_xyz[..., 2])            # [B, M] f32
    q64 = new_xyz.astype(np.float64)
    xn_full = ((xyz[..., 0] * xyz[..., 0] + xyz[..., 1] * xyz[..., 1])
               + xyz[..., 2] * xyz[..., 2])               # [B, N] f32
    # Pipeline the D2H with the rerank: each shard's fetch runs concurrently
    # in a worker thread (network wait releases the GIL); the per-batch rerank
    # happens in the same worker as its shard lands (in quarter-shard chunks
    # for finer GIL interleaving), hiding rerank compute behind the transfers.
    shards = sorted(out.addressable_shards, key=lambda s: s.index[0].start)
    res = np.empty((B, M, K), np.int32)
    C = M // 4

    def work(b):
        cand = np.asarray(shards[b].data)
        idx = cand.astype(np.int32)
        idx -= 1
        np.clip(idx, 0, N - 1, out=idx)
        for c0 in range(0, M, C):
            s = slice(c0, c0 + C)
            res[b, s] = _rerank_chunk(idx[s], xyz[b], xn_full[b],
                                      qn[b][s], q64[b][s])

    list(st.pool.map(work, range(B)))
    return res


# revision 22
# speedup vs baseline: 1.2749x; 1.0590x over previous
"""KNN top-32 kernel for Trainium2 (Bass/Tile), 8 NeuronCores.

Strategy:
  - Data-parallel over batch: core b handles batch element b (M=4096 queries,
    N=16384 database points, C=3).
  - Host ships raw coords only: qa3 = [2*q | x] as [3, M+N] f32 per core
    (1.97 MB total H2D). No augmented row is materialized: per query tile the
    PE computes s = 2*q.x - |x|^2 (a monotone-decreasing transform of the
    squared distance, per query row) as TWO accumulating K=3 fp32 matmuls
    into one PSUM tile: lhsT=(2q) (3 x 128) @ x (3 x 512), then
    lhsT=(-1s) (3 x 128) @ (x*x) (3 x 512), which adds -|x_n|^2 to every
    query row.
  - DVE reduces each 512-chunk with max8 (top-8 values) + max_index (their
    in-chunk indices) straight out of PSUM into a 256-wide table per 128-query
    tile. The true top-32 of a row is contained in the per-segment top-8 table
    (any segment holds at most 8 of a row's top-32 with overwhelming
    probability for randn inputs).
  - NROUNDS rounds of max8+match_replace(-BIG) on the table mark the top
    NCAND table slots; a compare+multiply turns the paired index table into a
    sparse key array (global_idx+1 at winners, 0 elsewhere), and NROUNDS more
    max8+match_replace rounds compact the candidate indices out, order-free.
    Output as uint16 (global_idx+1 fits 16 bits) to halve the D2H fetch.
  - Host re-ranks the candidates per query with bit-exact f32 reference
    arithmetic (single uint64 sort key: monotone f32 bits << 14 | idx) and
    emits the top-32 indices (int32).

Perf notes (axon-tunneled cores; wall-clock is network-dominated):
  - The jitted shard_map executable is built ONCE and cached; per-call cost is
    H2D of qa3, the NEFF execute (~ms), D2H of the u16 candidates.
  - Per-shard D2H fetches run in worker threads concurrently, and each batch
    element's rerank runs as its shard lands, hiding rerank behind transfers.
  - The per-segment offset table is generated on device with iota (no H2D).
  - The donated output buffers are created on device (no zeros upload).
"""

import numpy as np
from concurrent.futures import ThreadPoolExecutor

import jax
import jax.numpy as jnp
from jax.sharding import Mesh, NamedSharding, PartitionSpec
from jax.experimental.shard_map import shard_map

from concourse import bacc, bass2jax
import concourse.mybir as mybir
from concourse.tile import TileContext

B = 8
M = 4096          # queries per core
N = 16384         # database points per core
K = 32            # neighbors wanted
NROUNDS = 4
NCAND = 8 * NROUNDS  # candidates extracted per query
SEG = 512
G = N // SEG      # 32 segments -> table width 256
TW = G * 8        # table width
MT = 128          # query rows per tile
NT = M // MT      # 32 row tiles
NEG = -1.0e30

F32 = mybir.dt.float32
I32 = mybir.dt.int32
U16 = mybir.dt.uint16


def build_bass():
    nc = bacc.Bacc()
    qa3 = nc.declare_dram_parameter("qa3", [3, M + N], F32, isOutput=False)
    out = nc.declare_dram_parameter("out", [M, NCAND], U16, isOutput=True)

    with TileContext(nc) as tc, \
         tc.tile_pool(name="const", bufs=1) as cpool, \
         tc.tile_pool(name="work", bufs=2) as wpool, \
         tc.tile_pool(name="outp", bufs=3) as opool, \
         tc.tile_pool(name="psum", bufs=8, space="PSUM") as ppool:
        qx = cpool.tile([3, M + N], F32)
        nc.sync.dma_start(out=qx[:, :], in_=qa3[:, :])
        qs = qx[:, :M]
        asb = qx[:, M:]
        # database squares (for the -|x|^2 accumulation matmul)
        sq = cpool.tile([3, N], F32)
        nc.vector.tensor_mul(sq[:, :], asb[:, :], asb[:, :])
        # [3, MT] of -1: second matmul adds Sum_c -sq[c,n] = -|x_n|^2 to every
        # query row of the PSUM tile
        neg1 = cpool.tile([3, MT], F32)
        nc.vector.memset(neg1[:, :], -1.0)

        # per-slot global offset table: slot j -> (j//8)*SEG + 1 (same for all
        # partitions), generated on device instead of shipped over the tunnel
        ioff = cpool.tile([MT, TW], I32)
        nc.gpsimd.iota(ioff[:, :], [[SEG, G], [0, 8]], base=1,
                       channel_multiplier=0)
        offt = cpool.tile([MT, TW], F32)
        nc.vector.tensor_copy(offt[:, :], ioff[:, :])

        for t in range(NT):
            tbl = wpool.tile([MT, TW], F32, tag="tbl")
            idx16 = wpool.tile([MT, TW], U16, tag="idx16")
            for g in range(G):
                ps = ppool.tile([MT, SEG], F32, tag="ps")
                nc.tensor.matmul(
                    ps[:, :],
                    qs[:, t * MT:(t + 1) * MT],
                    asb[:, g * SEG:(g + 1) * SEG],
                    start=True,
                    stop=False,
                )
                nc.tensor.matmul(
                    ps[:, :],
                    neg1[:, :],
                    sq[:, g * SEG:(g + 1) * SEG],
                    start=False,
                    stop=True,
                )
                nc.vector.max(out=tbl[:, g * 8:(g + 1) * 8], in_=ps[:, :])
                nc.vector.max_index(
                    out=idx16[:, g * 8:(g + 1) * 8],
                    in_max=tbl[:, g * 8:(g + 1) * 8],
                    in_values=ps[:, :],
                )
            # paired global index table (value = global idx + 1) as f32
            idxf = wpool.tile([MT, TW], F32, tag="idxf")
            nc.vector.tensor_copy(idxf[:, :], idx16[:, :])
            nc.vector.tensor_add(idxf[:, :], idxf[:, :], offt[:, :])
            # pop top-NCAND values; winners' slots become NEG
            v8 = wpool.tile([MT, 8], F32, tag="v8")
            for r in range(NROUNDS):
                nc.vector.max(out=v8[:, :], in_=tbl[:, :])
                nc.vector.match_replace(
                    out=tbl[:, :], in_to_replace=v8[:, :], in_values=tbl[:, :],
                    imm_value=NEG,
                )
            # sparse key array: idx+1 where popped, 0 elsewhere
            wmask = wpool.tile([MT, TW], F32, tag="wmask")
            nc.vector.tensor_scalar(
                wmask[:, :], tbl[:, :], NEG, None, op0=mybir.AluOpType.is_equal
            )
            key = wpool.tile([MT, TW], F32, tag="key")
            nc.vector.tensor_mul(key[:, :], wmask[:, :], idxf[:, :])
            # compact the NCAND winning indices (order-free)
            outt = opool.tile([MT, NCAND], F32, tag="outt")
            for r in range(NROUNDS):
                nc.vector.max(out=outt[:, r * 8:(r + 1) * 8], in_=key[:, :])
                if r < NROUNDS - 1:
                    nc.vector.match_replace(
                        out=key[:, :], in_to_replace=outt[:, r * 8:(r + 1) * 8],
                        in_values=key[:, :], imm_value=0.0,
                    )
            out16 = opool.tile([MT, NCAND], U16, tag="out16")
            nc.vector.tensor_copy(out16[:, :], outt[:, :])
            nc.sync.dma_start(out=out[t * MT:(t + 1) * MT, :], in_=out16[:, :])
    nc.finalize()
    return nc


class _State:
    __slots__ = ("nc", "sharded", "zeros_jit", "pool", "next_zeros", "qa3_buf")


_STATE = None


def _get_state():
    global _STATE
    if _STATE is not None:
        return _STATE
    nc = build_bass()
    bass2jax.install_neuronx_cc_hook()

    partition_name = nc.partition_id_tensor.name if nc.partition_id_tensor else None
    in_names, out_names, out_avals = [], [], []
    for alloc in nc.m.functions[0].allocations:
        if not isinstance(alloc, mybir.MemoryLocationSet):
            continue
        name = alloc.memorylocations[0].name
        if alloc.kind == "ExternalInput":
            if name != partition_name:
                in_names.append(name)
        elif alloc.kind == "ExternalOutput":
            out_names.append(name)
            out_avals.append(jax.core.ShapedArray(
                tuple(alloc.tensor_shape), mybir.dt.np(alloc.dtype)))
    assert in_names == ["qa3"] and out_names == ["out"], (in_names, out_names)
    all_names = list(in_names + out_names)
    if partition_name is not None:
        all_names.append(partition_name)
    n_params = len(in_names)

    def _body(*args):
        operands = list(args)
        if partition_name is not None:
            operands.append(bass2jax.partition_id_tensor())
        outs = bass2jax._bass_exec_p.bind(
            *operands,
            out_avals=tuple(out_avals),
            in_names=tuple(all_names),
            out_names=tuple(out_names),
            lowering_input_output_aliases=(),
            sim_require_finite=True,
            sim_require_nnan=True,
            nc=nc,
        )
        return tuple(outs)

    devices = jax.devices()[:B]
    assert len(devices) == B, f"need {B} devices, got {len(jax.devices())}"
    mesh = Mesh(np.asarray(devices), ("core",))
    spec = PartitionSpec("core")
    sharded = jax.jit(
        shard_map(_body, mesh=mesh, in_specs=(spec,) * (n_params + 1),
                  out_specs=(spec,), check_rep=False),
        donate_argnums=(n_params,),
        keep_unused=True,
    )
    zeros_jit = jax.jit(
        lambda: jnp.zeros((B * M, NCAND), jnp.uint16),
        out_shardings=NamedSharding(mesh, spec),
    )
    st = _State()
    st.nc = nc
    st.sharded = sharded
    st.zeros_jit = zeros_jit
    st.pool = ThreadPoolExecutor(B)
    st.next_zeros = zeros_jit()
    st.qa3_buf = np.empty((B * 3, M + N), np.float32)
    _STATE = st
    return st


def _prep_qa3(xyz, new_xyz, buf):
    """Concatenated per-core raw-coord matrices: [B*3, M+N] f32 (into buf).

    Row block b: [2*qx, 2*qy, 2*qz | x, y, z] for batch element b. The query
    side carries the factor 2 (exact in fp32) so the device matmul computes
    s = 2*q.x - |x|^2 with the same rounding as scaling the database side.
    Reusing buf is safe: by the time kernel() returns, the previous call's
    H2D has been fully consumed (its outputs were fetched).
    """
    qa3 = buf.reshape(B, 3, M + N)
    np.multiply(new_xyz.transpose(0, 2, 1), np.float32(2.0), out=qa3[:, :, :M])
    qa3[:, :, M:] = xyz.transpose(0, 2, 1)
    return buf


def _rerank_chunk(idx, xyz_b, xn_b, qn_c, q64_c):
    """idx: [m, NCAND] int32 candidate indices (0-based, clipped) for a chunk
    of one batch element. xyz_b: [N, 3] f32; xn_b: [N] f32 |x|^2 table;
    qn_c: [m] f32 |q|^2; q64_c: [m, 3] f64 queries. Returns [m, K] int32.

    Re-ranks with bit-exact f32 reference arithmetic (XLA-CPU-matching
    rounding: fma emulated via f64 products). Sort key packs the monotone
    uint32 image of the f32 distance above the 14-bit index, so one uint64
    sort yields (dist asc, idx asc) — the reference's top_k tie order. When
    every distance in the chunk is >= 0 the raw IEEE bits are already
    monotone (and a sum ending in +xn can never round to -0.0), so the
    sign-flip mapping is skipped.
    """
    flat = idx.reshape(-1)
    x = np.take(xyz_b, flat, axis=0).reshape(*idx.shape, 3)

    x64 = x.astype(np.float64)
    t = np.multiply(q64_c[:, None, 0], x64[..., 0])
    acc = t.astype(np.float32)
    np.multiply(q64_c[:, None, 1], x64[..., 1], out=t)
    t += acc
    acc = t.astype(np.float32)
    np.multiply(q64_c[:, None, 2], x64[..., 2], out=t)
    t += acc
    acc = t.astype(np.float32)
    xn = np.take(xn_b, flat).reshape(idx.shape)
    d = ((np.float32(-2.0) * acc) + qn_c[:, None]).astype(np.float32) + xn
    d = np.ascontiguousarray(d, dtype=np.float32)

    bits = d.view(np.uint32)
    if d.min() >= 0.0:
        key = bits
    else:
        key = np.where(bits & np.uint32(0x80000000),
                       np.invert(bits), bits | np.uint32(0x80000000))
    comb = (key.astype(np.uint64) << np.uint64(14)) | idx.astype(np.uint64)
    comb.sort(axis=-1)
    return (comb[..., :K] & np.uint64(0x3FFF)).astype(np.int32)


def kernel(xyz, new_xyz):
    xyz = np.ascontiguousarray(xyz, dtype=np.float32)
    new_xyz = np.ascontiguousarray(new_xyz, dtype=np.float32)
    st = _get_state()
    qa3 = _prep_qa3(xyz, new_xyz, st.qa3_buf)
    try:
        out, = st.sharded(qa3, st.next_zeros)
    except Exception:
        # the donated buffer may be gone after a failed dispatch — retry once
        st.next_zeros = st.zeros_jit()
        out, = st.sharded(qa3, st.next_zeros)
    # rebuild the donated buffer for the next call while this one's
    # transfers are in flight
    st.next_zeros = st.zeros_jit()
    # candidate-independent rerank inputs, computed during the ~90 ms network
    # wait: query norms/f64 copies and the full database |x|^2 table (gathered
    # per candidate later instead of recomputed from coords)
    qn = ((new_xyz[..., 0] * new_xyz[..., 0] + new_xyz[..., 1] * new_xyz[..., 1])
          + new_xyz[..., 2] * new_xyz[..., 2])            # [B, M] f32
    q64 = new_xyz.astype(np.float64)
    xn_full = ((xyz[..., 0] * xyz[..., 0] + xyz[..., 1] * xyz[..., 1])
               + xyz[..., 2] * xyz[..., 2])               # [B, N] f32
    # Pipeline the D2H with the rerank: each shard's fetch runs concurrently
    # in a worker thread (network wait releases the GIL); the per-batch rerank
    # happens in the same worker as its shard lands (in quarter-shard chunks
    # for finer GIL interleaving), hiding rerank compute behind the transfers.
    shards = sorted(out.addressable_shards, key=lambda s: s.index[0].start)
    res = np.empty((B, M, K), np.int32)
    C = M // 4

    def work(b):
        cand = np.asarray(shards[b].data)
        idx = cand.astype(np.int32)
        idx -= 1
        np.clip(idx, 0, N - 1, out=idx)
        for c0 in range(0, M, C):
            s = slice(c0, c0 + C)
            res[b, s] = _rerank_chunk(idx[s], xyz[b], xn_full[b],
                                      qn[b][s], q64[b][s])

    list(st.pool.map(work, range(B)))
    return res
